# revision 26
# baseline (speedup 1.0000x reference)
"""Trainium2 Bass kernel for GCNN message passing.

out[b] = relu((A @ x[b]) @ W + bias),  A sparse [N, N] from 800k edges.

Sharding (8 NeuronCores): core h owns output rows [h*6272, (h+1)*6272) for
ALL 4 batches. Host interleaves x into xcat[n] = x[:, n, :] (bf16,
[N, 4*128]) so ONE gather descriptor fetches a neighbor's features for all
4 batches at once (Q7 descriptor generation is the bottleneck resource, at
~8ns per gather index).

Device algorithm per core:
  Host pre-sorts the core's ~100k edges by destination row into 25
  row-blocks of 256 rows; within a block edges are split into "low"
  (col < 32768) / "high" groups so gather indices fit in int16, padded to
  uniform L / H tiles of 128 edges (col=0/val=0 padding).
  The scaled one-hot scatter matrices S[e, r] = (r == rl[e]) * val[e]
  ([128, 256] bf16 per edge-tile) are PREBUILT ON HOST and streamed in
  (DMA has headroom).
  Per row-block:
    - two dma_gather ops (bases xcat[0:], xcat[32768:]) fetch
      msgs [128(edge), T, 512] bf16; edge slot k -> partition k%128,
      tile k//128.
    - PE accumulates aggT_b[c, r] += msgs[:, t, b*128:+128].T @ S_t into
      PSUM [128, 4*256] f32 (segment sum via matmul accumulation).
    - aggT -> SBUF bf16, PE applies W (outT_b = W.T @ aggT_b) into a
      second PSUM tile, ACT applies relu(.+bias), batched DMA writes
      outT [4, 128, 6400] f32.
  Host transposes/concatenates the 8 per-core outputs.
"""
import sys

import numpy as np

try:  # concourse (Bass) lives in the trn repo
    import concourse  # noqa: F401
except ImportError:  # pragma: no cover
    sys.path.insert(0, "/opt/trn_rl_repo")

import ml_dtypes

B, N, E, C = 4, 50000, 800000, 128
LAST_RESULTS = None  # BassKernelResults of the most recent kernel() call
P = 128
BR = 256            # rows per block
RB = 25             # row-blocks per core (covers 6400 >= 6272 rows)
RH = 6272           # row stride between cores (8 * 6272 = 50176 >= N)
NCORES = 8
SPLIT = 32768       # low/high column split for int16 gather indices
OUT_DMA_BLKS = 4    # row-blocks per output DMA


def _pack_idx(vals, n_slots):
    """dma_gather int16 index layout: index k at [k % 16, k // 16],
    replicated to 128 partitions; 0-padded. -> [128, n_slots // 16]"""
    buf = np.zeros(n_slots, np.int16)
    buf[:len(vals)] = vals
    tile16 = buf.reshape(n_slots // 16, 16).T
    return np.tile(tile16, (8, 1))


def _preprocess(edge_row, edge_col, edge_vals):
    """Per-core gather-index tables and host-built S matrices.

    Returns (lowidx [8, RB, 128, 8L], highidx [8, RB, 128, 8H],
             smat [8, 128, RB*T*BR] bf16, L, H).
    Edge slot k of a block: partition k%128, tile k//128; slots < L*128
    low-group (col), the rest high-group (col - SPLIT).
    S tile t of block blk lives at smat[:, (blk*T+t)*BR:(blk*T+t+1)*BR].
    """
    per_core = []
    maxlow = maxhigh = 0
    for h in range(NCORES):
        lo, hi = h * RH, min((h + 1) * RH, N)
        m = (edge_row >= lo) & (edge_row < hi)
        r, c, v = edge_row[m] - lo, edge_col[m], edge_vals[m]
        is_high = c >= SPLIT
        order = np.lexsort((is_high, r // BR))
        r, c, v, is_high = r[order], c[order], v[order], is_high[order]
        blocks = []
        for blk in range(RB):
            sel = slice(*np.searchsorted(r // BR, [blk, blk + 1]))
            rb, cb, vb, hb = r[sel], c[sel], v[sel], is_high[sel]
            nlow = int((~hb).sum())
            blocks.append((rb, cb, vb, nlow))
            maxlow = max(maxlow, nlow)
            maxhigh = max(maxhigh, len(rb) - nlow)
        per_core.append(blocks)
    L = (maxlow + P - 1) // P
    H = (maxhigh + P - 1) // P
    T = L + H
    lowidx = np.zeros((NCORES, RB, P, 8 * L), np.int16)
    highidx = np.zeros((NCORES, RB, P, 8 * H), np.int16)
    smat = np.zeros((NCORES, P, RB * T * BR), ml_dtypes.bfloat16)
    iota = np.arange(BR, dtype=np.float32)
    for h in range(NCORES):
        for blk in range(RB):
            rb, cb, vb, nlow = per_core[h][blk]
            nh = len(rb) - nlow
            lowidx[h, blk] = _pack_idx(cb[:nlow], L * P)
            highidx[h, blk] = _pack_idx(cb[nlow:] - SPLIT, H * P)
            rr = np.zeros(T * P, np.float32)
            vv = np.zeros(T * P, np.float32)
            rr[:nlow] = (rb[:nlow] - blk * BR).astype(np.float32)
            vv[:nlow] = vb[:nlow]
            rr[L * P:L * P + nh] = (rb[nlow:] - blk * BR).astype(np.float32)
            vv[L * P:L * P + nh] = vb[nlow:]
            # S[e, r] for slot e=t*P+p -> smat[p, (blk*T+t)*BR + r]
            s_f32 = (iota[None, :] == rr[:, None]) * vv[:, None]  # [T*P, BR]
            smat[h, :, blk * T * BR:(blk + 1) * T * BR] = (
                s_f32.reshape(T, P, BR).transpose(1, 0, 2).reshape(P, T * BR)
                .astype(ml_dtypes.bfloat16))
    return lowidx, highidx, smat, L, H


def _build_program(L, H, n_blocks=RB, n_rows=N):
    import concourse.bacc as bacc
    import concourse.tile as tile
    from concourse import mybir
    from concourse._compat import get_trn_type

    T = L + H
    BC = B * C                       # 512 feature cols in xcat
    f32 = mybir.dt.float32
    bf16 = mybir.dt.bfloat16
    i16 = mybir.dt.int16
    nc = bacc.Bacc(get_trn_type() or "TRN2", target_bir_lowering=False)

    x_d = nc.dram_tensor("xcat", [n_rows, BC], bf16, kind="ExternalInput")
    lowidx_d = nc.dram_tensor("lowidx", [P, n_blocks * 8 * L], i16,
                              kind="ExternalInput")
    highidx_d = nc.dram_tensor("highidx", [P, n_blocks * 8 * H], i16,
                               kind="ExternalInput")
    smat_d = nc.dram_tensor("smat", [P, n_blocks * T * BR], bf16,
                            kind="ExternalInput")
    wt_d = nc.dram_tensor("wt", [C, C], bf16, kind="ExternalInput")
    bias_d = nc.dram_tensor("bias", [C, 1], f32, kind="ExternalInput")
    out_d = nc.dram_tensor("outT", [B, C, n_blocks * BR], f32,
                           kind="ExternalOutput")

    with tile.TileContext(nc) as tc:
        with (
            tc.tile_pool(name="const", bufs=1) as const_pool,
            tc.tile_pool(name="meta", bufs=1) as meta_pool,
            tc.tile_pool(name="msgs", bufs=3) as msgs_pool,
            tc.tile_pool(name="smat", bufs=3) as s_pool,
            tc.tile_pool(name="aggsb", bufs=2) as agg_pool,
            tc.tile_pool(name="ostage", bufs=2) as ostage_pool,
            tc.tile_pool(name="psum_agg", bufs=2, space="PSUM") as psA,
            tc.tile_pool(name="psum_out", bufs=2, space="PSUM") as psO,
        ):
            wt_sb = const_pool.tile([C, C], bf16)
            bias_sb = const_pool.tile([C, 1], f32)
            nc.sync.dma_start(out=wt_sb[:], in_=wt_d[:])
            nc.sync.dma_start(out=bias_sb[:], in_=bias_d[:])

            lowidx_sb = meta_pool.tile([P, n_blocks * 8 * L], i16)
            highidx_sb = meta_pool.tile([P, n_blocks * 8 * H], i16)
            nc.sync.dma_start(out=lowidx_sb[:], in_=lowidx_d[:])
            nc.sync.dma_start(out=highidx_sb[:], in_=highidx_d[:])

            ostage = None
            for blk in range(n_blocks):
                msgs = msgs_pool.tile([P, T, BC], bf16)
                nc.gpsimd.dma_gather(
                    out_ap=msgs[:, :L, :],
                    in_ap=x_d[:SPLIT, :],
                    idxs_ap=lowidx_sb[:, blk * 8 * L:(blk + 1) * 8 * L],
                    num_idxs=L * P,
                    num_idxs_reg=L * P,
                    elem_size=BC,
                    single_packet=False,
                )
                nc.gpsimd.dma_gather(
                    out_ap=msgs[:, L:, :],
                    in_ap=x_d[SPLIT:, :],
                    idxs_ap=highidx_sb[:, blk * 8 * H:(blk + 1) * 8 * H],
                    num_idxs=H * P,
                    num_idxs_reg=H * P,
                    elem_size=BC,
                    single_packet=False,
                )
                s_blk = s_pool.tile([P, T * BR], bf16)
                nc.sync.dma_start(
                    out=s_blk[:],
                    in_=smat_d[:, blk * T * BR:(blk + 1) * T * BR])
                aggT_ps = psA.tile([C, B * BR], f32)
                for bb in range(B):
                    for t in range(T):
                        nc.tensor.matmul(
                            out=aggT_ps[:, bb * BR:(bb + 1) * BR],
                            lhsT=msgs[:, t, bb * C:(bb + 1) * C],
                            rhs=s_blk[:, t * BR:(t + 1) * BR],
                            start=(t == 0), stop=(t == T - 1),
                        )
                aggT_sb = agg_pool.tile([C, B * BR], bf16)
                nc.vector.tensor_copy(out=aggT_sb[:], in_=aggT_ps[:])
                outT_ps = psO.tile([C, B * BR], f32)
                for bb in range(B):
                    nc.tensor.matmul(
                        out=outT_ps[:, bb * BR:(bb + 1) * BR],
                        lhsT=wt_sb[:],
                        rhs=aggT_sb[:, bb * BR:(bb + 1) * BR],
                        start=True, stop=True)
                if blk % OUT_DMA_BLKS == 0:
                    ostage = ostage_pool.tile([C, B, OUT_DMA_BLKS * BR], f32)
                o_off = (blk % OUT_DMA_BLKS) * BR
                for bb in range(B):
                    nc.scalar.activation(
                        out=ostage[:, bb, o_off:o_off + BR],
                        in_=outT_ps[:, bb * BR:(bb + 1) * BR],
                        func=mybir.ActivationFunctionType.Relu,
                        bias=bias_sb[:, :1], scale=1.0,
                    )
                if blk % OUT_DMA_BLKS == OUT_DMA_BLKS - 1 or blk == n_blocks - 1:
                    lo_blk = (blk // OUT_DMA_BLKS) * OUT_DMA_BLKS
                    width = (blk - lo_blk + 1) * BR
                    for bb in range(B):
                        nc.sync.dma_start(
                            out=out_d[bb, :, lo_blk * BR: lo_blk * BR + width],
                            in_=ostage[:, bb, :width],
                        )
    return nc


def _ensure_ntff_hook_importable():
    """bass_utils imports antenv.axon_hooks when BASS_TRACE is set; this
    image lacks that module. Provide a null hook so tracing degrades
    gracefully instead of crashing."""
    import types

    try:
        import antenv.axon_hooks  # noqa: F401
        return
    except ImportError:
        pass
    mod = types.ModuleType("antenv.axon_hooks")
    mod.get_axon_ntff_profile_hook = lambda: None
    mod.set_axon_ntff_profile_hook = lambda h: None
    sys.modules["antenv.axon_hooks"] = mod
    try:
        import antenv
        antenv.axon_hooks = mod
    except ImportError:
        pass


def kernel(x, edge_row, edge_col, edge_vals, W, b):
    _ensure_ntff_hook_importable()
    from concourse.bass_utils import run_bass_kernel_spmd

    x = np.asarray(x, np.float32)
    edge_row = np.asarray(edge_row, np.int32)
    edge_col = np.asarray(edge_col, np.int32)
    edge_vals = np.asarray(edge_vals, np.float32)
    W = np.asarray(W, np.float32)
    b = np.asarray(b, np.float32)

    lowidx, highidx, smat, L, H = _preprocess(edge_row, edge_col, edge_vals)
    nc = _build_program(L, H)
    nc.compile()

    # xcat[n] = x[:, n, :] flattened -> [N, 4*128] bf16
    xcat = np.ascontiguousarray(
        x.transpose(1, 0, 2).reshape(N, B * C)).astype(ml_dtypes.bfloat16)
    wt = W.astype(ml_dtypes.bfloat16)
    in_maps = []
    for h in range(NCORES):
        in_maps.append({
            "xcat": xcat,
            "lowidx": np.ascontiguousarray(
                lowidx[h].transpose(1, 0, 2).reshape(P, RB * 8 * L)),
            "highidx": np.ascontiguousarray(
                highidx[h].transpose(1, 0, 2).reshape(P, RB * 8 * H)),
            "smat": smat[h],
            "wt": wt,
            "bias": np.ascontiguousarray(b[:, None]),
        })

    res = run_bass_kernel_spmd(nc, in_maps, list(range(NCORES)))
    global LAST_RESULTS
    LAST_RESULTS = res

    out = np.empty((B, N, C), np.float32)
    for h in range(NCORES):
        lo, hi = h * RH, min((h + 1) * RH, N)
        o = res.results[h]["outT"]              # [B, C, RB*BR]
        for bb in range(B):
            out[bb, lo:hi] = o[bb].T[:hi - lo]
    return out
